# revision 32
# baseline (speedup 1.0000x reference)
"""GCNConv kernel for 8x Trainium2 NeuronCores (Bass/Tile), v2.

Reference computation:
    h = x @ W + b                  # [N, 256] @ [256, 128] -> [N, 128]
    out[i] = sum_{e: dst[e]=i} val[e] * h[src[e]]

Strategy (per core; SPMD - one program, per-core data):
  - dst nodes sharded 12500/core (output rows).  Edges partitioned by dst.
  - Bias folded in as a "virtual node" per gather window: h[w*blkr+WREAL]=b,
    plus one virtual edge (src=bias row, dst=i, val=sum of vals into i) per
    dst node, assigned to window i%nblk.
  - Phase 1: every core computes the full h (fp16) into its own DRAM via
    PE matmuls.  Host permutes the projection's row-stream order so each
    SBUF partition holds two CONSECUTIVE h rows -> 512B DMA descriptors.
  - Phase 2: per-edge h rows pulled on-chip with dma_gather (int16 indices,
    5 source windows of 20608 rows).  Per 128-edge chunk, a staircase
    matrix B [128e x 32seg] fp16 (carrying val) is built ON-CHIP from
    per-slot (val, segid) arrays via iota+is_equal, and used as the
    stationary matmul operand -> PSUM partial segment sums.  A second
    on-chip one-hot S2 [128seg x 128dst] (from per-(row,mm) dl arrays)
    accumulates segments into per-dst-tile PSUM, evicted to DRAM fp32.
  - Chunks are bucketed per (group of TPG dst tiles, window); a chunk's
    segments may span several tiles of its group (S2 routes them).
All data-dependent structure is padded/unioned to the max across cores so
the same program serves all 8 cores.
"""

import sys

for _p in ("/opt/trn_rl_repo",):
    if _p not in sys.path:
        sys.path.insert(0, _p)

import numpy as np

P = 128
MSEG = 32            # segment slots per 128-edge chunk
TPG = 6              # dst tiles (of 128 dst) per processing group
                     # (each open PSUM accumulator needs its own 2KB bank:
                     #  TPG l2 banks + 2 psL1 banks <= 8)
PSG = 16             # chunks per psum-group (ps1 = [128, PSG*MSEG] f32)
N_CORES = 8
WREAL = 20480        # real rows per gather window
RB = 512             # projection psum row-batch
XB = 2560            # projection DMA row-batch (= WREAL/8: one
                     # per-core shard of each window)


def _ceil_to(a, m):
    return -(-a // m) * m


class Plan:
    """Static (core-invariant) program structure + per-core data arrays."""


def build_plan(x, edge_src, edge_dst, edge_vals, weight, bias):
    N, IN_F = x.shape
    OUT_F = weight.shape[1]
    assert N % N_CORES == 0
    ndst = N // N_CORES                    # dst nodes per core
    ndst_pad = _ceil_to(ndst, P)
    ntile = ndst_pad // P                  # dst tiles per core
    ngrp = -(-ntile // TPG)
    nblk = -(-N // WREAL)                  # gather windows
    blkr = WREAL                           # h rows per window
    assert blkr <= 32767
    hrows = nblk * blkr                    # total h rows
    ncols = nblk * WREAL                   # projection column count
    assert ncols % XB == 0 and WREAL % XB == 0

    pl = Plan()
    pl.N, pl.IN_F, pl.OUT_F = N, IN_F, OUT_F
    pl.ndst, pl.ndst_pad, pl.ntile, pl.ngrp = ndst, ndst_pad, ntile, ngrp
    pl.hrows, pl.nblk, pl.blkr, pl.ncols = hrows, nblk, blkr, ncols
    pl.kc = IN_F // P                      # K chunks for projection

    # --- projection stream: permuted so partition p holds row pairs ---
    # column q of the stream maps to in-window row:
    #   (q%XB): sub = u2//RB, u = u2%RB, w256 = u//256, v = u%256
    #   inwin = (j%perwin)*XB + sub*RB + w256*256 + 2*(v%128) + v//128
    q = np.arange(ncols, dtype=np.int64)
    perwin = WREAL // XB
    j = q // XB
    u2 = q % XB
    u = u2 % RB
    v = u % 256
    inwin = (j % perwin) * XB + (u2 // RB) * RB + (u // 256) * 256 \
        + 2 * (v % 128) + v // 128
    xrow = (j // perwin) * WREAL + inwin
    valid = xrow < N
    xs = np.zeros((ncols, IN_F), np.float16)
    xs[valid] = x.astype(np.float16)[xrow[valid]]
    xT_full = np.ascontiguousarray(
        xs.T.reshape(pl.kc, P, ncols))           # [kc, P, ncols]
    # shard: core ci owns batch ci of each window (XB cols per window)
    assert perwin == N_CORES
    sel = (q // XB) % N_CORES
    pl.xT = np.stack([np.ascontiguousarray(xT_full[:, :, sel == ci])
                      for ci in range(N_CORES)])  # [N_CORES, kc, P, ncols/8]
    pl.W = np.ascontiguousarray(
        weight.astype(np.float16).reshape(pl.kc, P, OUT_F).transpose(1, 0, 2)
    )  # [P, kc, OUT_F]
    pl.bvec = np.ascontiguousarray(bias.astype(np.float16)[None, :])

    # --- edges; bias handled as a rank-1 deg_w x b update on-device ---
    deg_w = np.bincount(edge_dst, weights=edge_vals.astype(np.float64),
                        minlength=N).astype(np.float32)
    degp = np.zeros((N_CORES, 1, ndst_pad), np.float16)
    degp[:, 0, :ndst] = deg_w.reshape(N_CORES, ndst)
    pl.degp = degp
    dst_a = edge_dst.astype(np.int64)
    blk_a = edge_src.astype(np.int64) // WREAL
    pos_a = edge_src.astype(np.int64) % WREAL
    val_a = edge_vals.astype(np.float32)

    core = dst_a // ndst
    dl = dst_a % ndst
    tile = dl // P
    grp = tile // TPG
    order = np.lexsort((dl, blk_a, grp, core))
    dst_a, blk_a, pos_a, val_a = (dst_a[order], blk_a[order], pos_a[order],
                                  val_a[order])
    core, dl, tile, grp = core[order], dl[order], tile[order], grp[order]

    # run = consecutive edges with same (core, grp, blk, dl)
    key_change = np.ones(len(dst_a), bool)
    if len(dst_a) > 1:
        key_change[1:] = ((core[1:] != core[:-1]) | (grp[1:] != grp[:-1]) |
                          (blk_a[1:] != blk_a[:-1]) | (dl[1:] != dl[:-1]))
    run_starts = np.nonzero(key_change)[0]
    run_lens = np.diff(np.append(run_starts, len(dst_a)))
    r_core = core[run_starts]
    r_grp = grp[run_starts]
    r_blk = blk_a[run_starts]
    r_dl = dl[run_starts]

    # --- greedy chunk/segment layout per merged bucket (core, grp, blk) ---
    nbuck = ngrp * nblk

    def bucket_id(g, b):
        return g * nblk + b

    def greedy(lens):
        pieces = []
        c, s, d = 0, 0, 0
        for ri, ln in enumerate(lens):
            rem = ln
            while rem > 0:
                if s == P or d == MSEG:
                    c += 1
                    s, d = 0, 0
                take = min(P - s, rem)
                pieces.append((ri, take, c, s, d))
                s += take
                d += 1
                rem -= take
        return pieces, (c + 1 if (s > 0 or c == 0) else c)

    rb = (r_core * nbuck + bucket_id(r_grp, r_blk)).astype(np.int64)
    rb_order = np.argsort(rb, kind="stable")
    chunks_cb = np.zeros((N_CORES, nbuck), np.int64)
    bucket_pieces = {}
    i = 0
    rb_sorted = rb[rb_order]
    while i < len(rb_sorted):
        jj = i
        while jj < len(rb_sorted) and rb_sorted[jj] == rb_sorted[i]:
            jj += 1
        ridx = rb_order[i:jj]
        cb = int(rb_sorted[i])
        pieces, nch = greedy(run_lens[ridx])
        bucket_pieces[cb] = (ridx, pieces)
        chunks_cb[cb // nbuck, cb % nbuck] = nch
        i = jj

    chunks_b = chunks_cb.max(axis=0)       # static per-bucket chunk count
    grp_tiles = [list(range(g * TPG, min((g + 1) * TPG, ntile)))
                 for g in range(ngrp)]
    for g in range(ngrp):
        bids = [bucket_id(g, b) for b in range(nblk)]
        tot = int(sum(chunks_b[b] for b in bids))
        pad = _ceil_to(max(tot, 4), 4) - tot
        chunks_b[bucket_id(g, nblk - 1)] += pad

    chunk_off_b = np.zeros(nbuck, np.int64)
    off = 0
    grp_chunk_off = []
    for g in range(ngrp):
        grp_chunk_off.append(off)
        for b in range(nblk):
            bid = bucket_id(g, b)
            chunk_off_b[bid] = off
            off += int(chunks_b[bid])
    CC = off
    grp_chunk_off.append(CC)
    TOT = CC * P

    gather_sizes = np.zeros((ngrp, nblk), np.int64)
    gather_off = np.zeros((ngrp, nblk), np.int64)
    for g in range(ngrp):
        for b in range(nblk):
            gather_sizes[g, b] = int(chunks_b[bucket_id(g, b)]) * P
            gather_off[g, b] = int(chunk_off_b[bucket_id(g, b)]) * P

    # --- fill per-core slot arrays ---
    slot_src = np.zeros((N_CORES, TOT), np.int16)    # idx within window
    Bf = np.zeros((N_CORES, P, CC * MSEG), np.float16)
    seg_chunk, seg_slot, seg_dl, seg_core = [], [], [], []
    for cb, (ridx, pieces) in bucket_pieces.items():
        ci, bid = cb // nbuck, cb % nbuck
        base_c = int(chunk_off_b[bid])
        pr = np.array([p[0] for p in pieces])
        pt = np.array([p[1] for p in pieces])
        pc = np.array([p[2] for p in pieces]) + base_c
        ps_ = np.array([p[3] for p in pieces])
        pd = np.array([p[4] for p in pieces])
        gri = ridx[pr]
        src_off = np.zeros(len(pieces), np.int64)
        for k in range(1, len(pieces)):
            if pr[k] == pr[k - 1]:
                src_off[k] = src_off[k - 1] + pt[k - 1]
        e_start = run_starts[gri] + src_off
        slot_start = pc * P + ps_
        rep = np.repeat(np.arange(len(pieces)), pt)
        within = np.arange(len(rep)) - np.repeat(
            np.concatenate([[0], np.cumsum(pt)[:-1]]), pt)
        e_idx = e_start[rep] + within
        sl_idx = slot_start[rep] + within
        slot_src[ci, sl_idx] = pos_a[e_idx].astype(np.int16)
        Bf[ci, sl_idx % P, (sl_idx // P) * MSEG + pd[rep]] = \
            val_a[e_idx].astype(np.float16)
        seg_chunk.append(pc)
        seg_slot.append(pd)
        seg_dl.append(r_dl[gri])
        seg_core.append(np.full(len(pieces), ci))

    seg_chunk = np.concatenate(seg_chunk)
    seg_slot = np.concatenate(seg_slot)
    seg_dl = np.concatenate(seg_dl)
    seg_core = np.concatenate(seg_core)

    # --- L2 program: union over cores of (L2 chunk, tile) pairs ---
    s_j = seg_chunk // 4
    s_tile = seg_dl // P
    jt = np.unique(s_j * ntile + s_tile)
    l2_mms = [(int(v // ntile), int(v % ntile)) for v in jt]   # sorted (j,t)
    NMM = len(l2_mms)
    mm_index = {jt_: i for i, jt_ in enumerate(l2_mms)}
    mm_start = np.zeros(NMM, bool)
    mm_stop = np.zeros(NMM, bool)
    seen = {}
    for i, (jv, t) in enumerate(l2_mms):
        g = next(gg for gg in range(ngrp)
                 if grp_chunk_off[gg] <= 4 * jv < grp_chunk_off[gg + 1])
        if (g, t) not in seen:
            mm_start[i] = True
        seen[(g, t)] = i
    for (g, t), i in seen.items():
        mm_stop[i] = True

    # dl-or-(-1) per (seg row, mm): [N_CORES, P, NMM]
    dl_arr = np.full((N_CORES, P, NMM), -1, np.int16)
    s_mm = np.array([mm_index[(int(jv), int(t))]
                     for jv, t in zip(s_j, s_tile)])
    s_row = (seg_chunk % 4) * MSEG + seg_slot
    dl_arr[seg_core, s_row, s_mm] = (seg_dl % P).astype(np.int16)

    # MAXMM: max mms per psum-group (j in [j0, j0+4))
    n_pg = CC // PSG
    mm_j = np.array([jv for jv, _ in l2_mms])
    MAXMM = 1
    for pg in range(n_pg):
        j0 = pg * PSG // 4
        MAXMM = max(MAXMM, int(((mm_j >= j0) & (mm_j < j0 + 4)).sum()))

    # idx tensor: per gather call, slot q -> [q % 16, off16 + q // 16],
    # replicated 8x across the 128 partitions
    IDX = np.zeros((N_CORES, 16, TOT // 16), np.int16)
    for g in range(ngrp):
        for b in range(nblk):
            o, n = int(gather_off[g, b]), int(gather_sizes[g, b])
            if n == 0:
                continue
            IDX[:, :, o // 16:(o + n) // 16] = slot_src[
                :, o:o + n].reshape(N_CORES, n // 16, 16).transpose(0, 2, 1)
    IDX = np.tile(IDX, (1, 8, 1))          # -> [N_CORES, 128, TOT // 16]

    pl.chunks_b, pl.chunk_off_b = chunks_b, chunk_off_b
    pl.grp_tiles, pl.grp_chunk_off = grp_tiles, grp_chunk_off
    pl.CC, pl.TOT, pl.NMM, pl.MAXMM = CC, TOT, NMM, MAXMM
    pl.gather_sizes, pl.gather_off = gather_sizes, gather_off
    pl.l2_mms, pl.mm_start, pl.mm_stop = l2_mms, mm_start, mm_stop
    pl.IDX, pl.Bf, pl.dl_arr = IDX, Bf, dl_arr
    return pl


# ---------------------------------------------------------------------------
# Device program
# ---------------------------------------------------------------------------

def build_bass(pl):
    import os
    REP1 = int(os.environ.get("K_REP1", "1"))   # phase-1 repeats (timing)
    REP2 = int(os.environ.get("K_REP2", "1"))   # phase-2 repeats (timing)
    GMAX = int(os.environ.get("K_GMAX", "8192"))
    NSWQ = int(os.environ.get("K_NSWQ", "4"))
    NOCC = int(os.environ.get("K_NOCC", "0"))   # stub collectives (sim only)
    TIMING = int(os.environ.get("K_TIMING", "0"))  # big inputs internal
    SKIP = set(filter(None, os.environ.get("K_SKIP", "").split(",")))
    import concourse.bass as bass
    import concourse.mybir as mybir
    import concourse.tile as tile
    from concourse import bacc

    f16 = mybir.dt.float16
    f32 = mybir.dt.float32
    i16 = mybir.dt.int16

    nc = bacc.Bacc("TRN2", target_bir_lowering=False, debug=False,
                   num_swdge_queues=NSWQ, num_devices=N_CORES)

    OF = pl.OUT_F
    big = "Internal" if TIMING else "ExternalInput"
    xT_d = nc.dram_tensor("xt", [pl.kc, P, pl.ncols // N_CORES], f16,
                          kind=big)
    W_d = nc.dram_tensor("w", [P, pl.kc, OF], f16, kind="ExternalInput")
    b_d = nc.dram_tensor("bvec", [1, OF], f16, kind="ExternalInput")
    idx_d = nc.dram_tensor("idx", [P, pl.TOT // 16], i16, kind=big)
    B_d = nc.dram_tensor("bmat", [P, pl.CC * MSEG], f16, kind=big)
    dl_d = nc.dram_tensor("dl", [P, pl.NMM], i16, kind=big)
    deg_d = nc.dram_tensor("deg", [1, pl.ndst_pad], f16, kind="ExternalInput")
    out_kind = "Internal" if TIMING else "ExternalOutput"
    out_d = nc.dram_tensor("out", [pl.ndst_pad, OF], f32, kind=out_kind)
    if TIMING:
        dum_d = nc.dram_tensor("tdum", [1, OF], f32, kind="ExternalOutput")
    h_ds = [nc.dram_tensor(f"hbuf{b}", [pl.blkr, OF], f16)
            for b in range(pl.nblk)]
    hstg_ds = [nc.dram_tensor(f"hstg{b}", [XB, OF], f16)
               for b in range(pl.nblk)]

    # static per-group mm ranges
    grp_mm_lo = []
    _mm = 0
    for g in range(pl.ngrp):
        grp_mm_lo.append(_mm)
        j_hi = pl.grp_chunk_off[g + 1] // 4
        while _mm < pl.NMM and pl.l2_mms[_mm][0] < j_hi:
            _mm += 1
    grp_mm_lo.append(pl.NMM)

    with tile.TileContext(nc) as tc:
        with (
            tc.tile_pool(name="pconst", bufs=1) as pconst,
            tc.tile_pool(name="pxt", bufs=2) as pxt,
            tc.tile_pool(name="phs", bufs=2) as phs,
            tc.tile_pool(name="pidx", bufs=2) as pidx,
            tc.tile_pool(name="pmsg", bufs=6) as pmsg,
            tc.tile_pool(name="pB", bufs=2) as pB,
            tc.tile_pool(name="pS2", bufs=2) as pS2,
            tc.tile_pool(name="pP", bufs=3) as pP,
            tc.tile_pool(name="pout", bufs=2) as pout,
            tc.tile_pool(name="pps", bufs=2, space="PSUM") as pps,
            tc.tile_pool(name="psL2", bufs=TPG, space="PSUM") as psL2,
        ):
            # constants + metadata, resident for the kernel lifetime
            W_sb = pconst.tile([P, pl.kc, OF], f16)
            nc.sync.dma_start(W_sb[:], W_d[:])
            b_sb = pconst.tile([1, OF], f16)
            nc.sync.dma_start(b_sb[:], b_d[:])
            dl_sb = pconst.tile([P, pl.NMM], i16)
            nc.scalar.dma_start(dl_sb[:], dl_d[:])
            iotaS = pconst.tile([P, P], i16)
            nc.gpsimd.iota(iotaS[:], pattern=[[1, P]],
                           channel_multiplier=0)
            deg_sb = pconst.tile([1, pl.ndst_pad], f16)
            nc.sync.dma_start(deg_sb[:], deg_d[:])

            # ------- Phase 1: sharded h = x @ W, allgather per window -------
            # core ci computes rows [ci*XB, (ci+1)*XB) of each window, then
            # an AllGather assembles the full window in every core's DRAM.
            for _rep1 in range(REP1):
                for wi in range(pl.nblk):
                    c0 = wi * XB
                    xt = pxt.tile([P, pl.kc, XB], f16, tag="xt")
                    nc.sync.dma_start(
                        xt[:],
                        xT_d[:, :, c0:c0 + XB].rearrange("k p c -> p k c"),
                    )
                    hs = phs.tile([P, XB], f16, tag="hs")
                    for sub in range(XB // RB):
                        ps = pps.tile([P, RB], f32, tag="ps")
                        for rc in range(RB // P):
                            q0 = sub * RB + rc * P
                            for k in range(pl.kc):
                                nc.tensor.matmul(
                                    ps[:, rc * P:(rc + 1) * P],
                                    lhsT=xt[:, k, q0:q0 + P],
                                    rhs=W_sb[:, k, :],
                                    start=(k == 0),
                                    stop=(k == pl.kc - 1),
                                )
                        nc.scalar.copy(
                            hs[:, sub * RB:(sub + 1) * RB], ps[:])
                    nc.sync.dma_start(
                        hstg_ds[wi][:, :].rearrange(
                            "(w p t) f -> p w (t f)", p=P, t=2),
                        hs[:].rearrange("p (w c) -> p w c", c=256),
                    )
                    if NOCC:
                        nc.sync.dma_start(h_ds[wi][0:XB, :],
                                          hstg_ds[wi][:, :])
                    else:
                        nc.gpsimd.collective_compute(
                            "AllGather",
                            mybir.AluOpType.bypass,
                            replica_groups=[list(range(N_CORES))],
                            ins=[hstg_ds[wi][:, :].opt()],
                            outs=[h_ds[wi][:, :].opt()],
                        )

            # ---------------- Phase 2: gather + L1 + L2 ----------------
            qrot = [0]
            for _rep2 in range(REP2):
                mm_i = 0          # global L2 mm counter
                for g in range(pl.ngrp):
                    tiles_g = pl.grp_tiles[g]
                    nt_g = len(tiles_g)
                    c_lo, c_hi = pl.grp_chunk_off[g], pl.grp_chunk_off[g + 1]
                    nch_g = c_hi - c_lo
                    o_g = c_lo * P
                    tot_g = nch_g * P
                    assert mm_i == grp_mm_lo[g]
                    mm_lo, mm_hi = grp_mm_lo[g], grp_mm_lo[g + 1]
                    nmm_g = mm_hi - mm_lo

                    ixt = pidx.tile([P, tot_g // 16], i16, tag="idx")
                    nc.scalar.dma_start(
                        ixt[:], idx_d[:, o_g // 16:(o_g + tot_g) // 16])
                    msgs = {}
                    for b in range(pl.nblk):
                        n = int(pl.gather_sizes[g, b])
                        if n == 0:
                            continue
                        o = int(pl.gather_off[g, b])
                        mt = pmsg.tile([P, n // P, OF], f16, tag="msg")
                        if "gather" in SKIP:
                            nc.vector.memset(mt[:, 0, 0:1], 0.0)
                        for q0 in range(0, n, GMAX):
                            if "gather" in SKIP:
                                continue
                            qn = min(GMAX, n - q0)
                            nc.gpsimd.dma_gather(
                                out_ap=mt[:, q0 // P:(q0 + qn) // P, :],
                                in_ap=h_ds[b][:, :],
                                idxs_ap=ixt[:, (o - o_g + q0) // 16:
                                            (o - o_g + q0 + qn) // 16],
                                num_idxs=qn,
                                num_idxs_reg=qn,
                                elem_size=OF,
                                single_packet=False,
                                queue_num=qrot[0] % nc.num_swdge_queues,
                            )
                            qrot[0] += 1
                        msgs[b] = (mt, o)

                    # B streamed from DRAM; S2 generated on-chip
                    Bt = pB.tile([P, nch_g, MSEG], f16, tag="B")
                    nc.sync.dma_start(
                        Bt[:].rearrange("p a b -> p (a b)"),
                        B_d[:, c_lo * MSEG:c_hi * MSEG])
                    s2t = None
                    if nmm_g:
                        s2t = pS2.tile([P, nmm_g, P], f16, tag="s2")
                        if "s2gen" in SKIP:
                            nc.vector.memset(s2t[:, 0, 0:1], 0.0)
                        else:
                            nc.vector.tensor_tensor(
                                s2t[:],
                                iotaS[:].unsqueeze(1)
                                .broadcast_to([P, nmm_g, P]),
                                dl_sb[:, mm_lo:mm_hi].unsqueeze(-1)
                                .broadcast_to([P, nmm_g, P]),
                                mybir.AluOpType.is_equal)

                    l2ps = {}
                    for t in tiles_g:
                        l2ps[t] = psL2.tile([P, OF], f32, tag="l2",
                                            name=f"l2ps_{g}_{t}")
                        if "l2mm" in SKIP:
                            nc.vector.memset(l2ps[t][:, 0:1], 0.0)

                    pg_sizes = [PSG] * (nch_g // PSG)
                    if nch_g % PSG:
                        pg_sizes.append(nch_g % PSG)
                    pg_off = 0
                    for npg_c in pg_sizes:
                        c0 = c_lo + pg_off
                        pg_off += npg_c
                        ps1 = pps.tile([P, npg_c * MSEG], f32, tag="ps")
                        if "l1mm" in SKIP:
                            nc.vector.memset(ps1[:, 0:1], 0.0)
                        for cc in range(npg_c):
                            c = c0 + cc
                            so = c * P
                            b = None
                            for bb in range(pl.nblk):
                                o = int(pl.gather_off[g, bb])
                                n = int(pl.gather_sizes[g, bb])
                                if o <= so < o + n:
                                    b = bb
                                    break
                            mt, o = msgs[b]
                            ci = (so - o) // P
                            cg = cc % 4
                            w = (cc // 4) % 4
                            if "l1mm" in SKIP:
                                continue
                            nc.tensor.matmul(
                                ps1[MSEG * cg:MSEG * (cg + 1),
                                    w * P:(w + 1) * P],
                                lhsT=Bt[:, c0 - c_lo + cc, :],
                                rhs=mt[:, ci, :],
                                start=True,
                                stop=True,
                                tile_position=(0, MSEG * cg),
                            )
                        Pt = pP.tile([P, npg_c // 4, OF], f16, tag="P")
                        if "ptcopy" in SKIP:
                            nc.vector.memset(Pt[:, 0, 0:1], 0.0)
                        else:
                            nc.scalar.copy(
                                Pt[:].rearrange("p a b -> p (a b)"), ps1[:])
                        j0 = c0 // 4
                        mms_here = []
                        while (mm_i < pl.NMM and
                               pl.l2_mms[mm_i][0] < j0 + npg_c // 4):
                            mms_here.append(mm_i)
                            mm_i += 1
                        for mi in mms_here:
                            if "l2mm" in SKIP:
                                continue
                            jv, t = pl.l2_mms[mi]
                            nc.tensor.matmul(
                                l2ps[t][:],
                                lhsT=s2t[:, mi - mm_lo, :],
                                rhs=Pt[:, jv - j0, :],
                                start=bool(pl.mm_start[mi]),
                                stop=False,
                            )
                    # bias: out[t] += deg_w (x) b, closes the accumulation
                    for t in tiles_g:
                        if "l2mm" in SKIP:
                            continue
                        nc.tensor.matmul(
                            l2ps[t][:],
                            lhsT=deg_sb[0:1, t * P:(t + 1) * P],
                            rhs=b_sb[0:1, :],
                            start=False,
                            stop=True,
                        )
                    # evict group's dst tiles
                    ot = pout.tile([P, nt_g, OF], f32, tag="out")
                    for ti, t in enumerate(tiles_g):
                        nc.scalar.copy(ot[:, ti, :], l2ps[t][:])
                    t0 = tiles_g[0]
                    nc.sync.dma_start(
                        out_d[t0 * P:(t0 + nt_g) * P, :].rearrange(
                            "(t p) f -> p t f", p=P),
                        ot[:])
            if TIMING:
                dt_ = pout.tile([1, OF], f32, tag="dum")
                nc.vector.memset(dt_[:], 0.0)
                nc.sync.dma_start(dum_d[:], dt_[:])

    nc.compile()
    return nc


# ---------------------------------------------------------------------------
# Entry point
# ---------------------------------------------------------------------------

def kernel(x, edge_src, edge_dst, edge_vals, weight, bias,
           _want_trace=False, _n_cores=None):
    x = np.asarray(x)
    edge_src = np.asarray(edge_src)
    edge_dst = np.asarray(edge_dst)
    edge_vals = np.asarray(edge_vals)
    weight = np.asarray(weight)
    bias = np.asarray(bias)

    pl = build_plan(x, edge_src, edge_dst, edge_vals, weight, bias)
    nc = build_bass(pl)

    from concourse.bass_utils import run_bass_kernel_spmd

    ncores = N_CORES if _n_cores is None else _n_cores
    in_maps = []
    for ci in range(ncores):
        in_maps.append({
            "xt": np.ascontiguousarray(pl.xT[ci]),
            "w": pl.W,
            "bvec": pl.bvec,
            "idx": np.ascontiguousarray(pl.IDX[ci]),
            "bmat": np.ascontiguousarray(pl.Bf[ci]),
            "deg": np.ascontiguousarray(pl.degp[ci]),
            "dl": np.ascontiguousarray(pl.dl_arr[ci]),
        })
    res = run_bass_kernel_spmd(nc, in_maps, core_ids=list(range(ncores)),
                               trace=_want_trace)
    outs = [res.results[ci]["out"][:pl.ndst, :] for ci in range(ncores)]
    if ncores < N_CORES:
        outs += [np.zeros((pl.ndst, pl.OUT_F), np.float32)] * (N_CORES - ncores)
    full = np.concatenate(outs, axis=0).astype(np.float32)
    if _want_trace:
        kernel._last_results = res
    return full
